# revision 16
# baseline (speedup 1.0000x reference)
"""Causal self-attention (B=2, T=4096, C=768, H=12) on 8 trn2 NeuronCores.

Sharding: data-parallel on batch (cores 0-3 -> batch 0, cores 4-7 -> batch 1),
tensor-parallel on heads (3 heads per core).  Each core computes qkv for its
3 heads, causal flash-style attention, and a partial output projection
(its heads' rows of w_proj); the host sums the 4 partials per batch.

v11 structure (vs the serial-phase v7 baseline, ~2.05x faster: 676us ->
330us per iteration measured via repeat-differencing with block sampling):
- All activations/weights in bf16 (host-converted): halves DMA traffic and
  removes every fp32->fp32r rounding copy.  PSUM accumulation stays fp32.
  Partial Y outputs are written bf16 and summed fp32 on the host.
- Causal masking via gpsimd affine_select directly on the exp'd P tile
  (Pool engine is otherwise idle), freeing DVE; diagonal tiles compute
  S/PV ragged (columns left of the diagonal tile are skipped).
- One software-pipelined loop: the qkv projection chunk qs+1, V^T->V
  transposes, and the output projection for qs-1 are emitted interleaved
  into the attention rotation for query superblock qs, so their DMA/PE/
  DVE work hides under the attention inner loop (PE ~83% busy in sim).
- x^T is host-swizzled to [partition, chunk, cchunk, token] so each chunk
  DMA is one contiguous 6KB run per partition; y writes batch 4 token
  tiles per DMA.
"""

import sys

if '/opt/trn_rl_repo' not in sys.path:
    sys.path.insert(0, '/opt/trn_rl_repo')

from collections import deque

import numpy as np
import ml_dtypes

import concourse.bacc as bacc
import concourse.mybir as mybir
import concourse.tile as tile
from concourse.masks import make_identity

dt = mybir.dt
F32 = dt.float32
BF16 = dt.bfloat16
FP8 = dt.float8e4
NP_BF16 = ml_dtypes.bfloat16

# exp bias (in log space) applied to every attention logit before the fp8
# P tile: keeps exp(max_logit)+margin under the TRN fp8e4 max of 240 while
# keeping every row's max P far above the subnormal flush threshold.  The
# uniform scale cancels in the softmax normalization.
EXP_BIAS = -3.0 * float(np.log(2.0))

N_EMBD = 768
N_HEADS = 12
HEAD_DIM = 64
B = 2
T_FULL = 4096
N_CORES = 8
HEADS_PER_CORE = N_HEADS // (N_CORES // B)  # 3

TOK_CHUNK = 512   # qkv phase token chunk == query superblock
QSB = 512         # attention query superblock
KT = 128          # key tile (contraction for P@V)
CCHUNKS = N_EMBD // 128  # 6 contraction chunks


def build_nc(T=T_FULL, repeat=1, phases=('B', 'B2', 'C', 'D')):
    """Build the per-core Bass program.  Same program runs SPMD on all 8
    cores; per-core data (x^T of its batch, its heads' weight slices) comes
    via the input map.  `phases` subsets the per-iteration work (timing
    ablation only -- outputs are garbage unless all phases run)."""
    nc = bacc.Bacc(None, target_bir_lowering=False, debug=False)

    n_kt = T // KT
    n_qsb = T // QSB
    n_tok = T // 128
    kt_per_qsb = QSB // KT  # 4

    # x^T pre-swizzled on host to [p, chunk, cchunk, tok]: each chunk DMA
    # reads one contiguous 6KB run per partition.
    XT = nc.dram_tensor(
        "xt", [128, T // TOK_CHUNK, CCHUNKS, TOK_CHUNK], BF16,
        kind="ExternalInput")
    WQ01 = nc.dram_tensor("wq01", [N_EMBD, 128], BF16, kind="ExternalInput")
    WK01 = nc.dram_tensor("wk01", [N_EMBD, 128], BF16, kind="ExternalInput")
    WV01 = nc.dram_tensor("wv01", [N_EMBD, 128], BF16, kind="ExternalInput")
    WQV2 = nc.dram_tensor("wqv2", [N_EMBD, 128], BF16, kind="ExternalInput")
    WK2 = nc.dram_tensor("wk2", [N_EMBD, 64], BF16, kind="ExternalInput")
    WP1 = nc.dram_tensor("wp1", [128, N_EMBD], BF16, kind="ExternalInput")
    WP2 = nc.dram_tensor("wp2", [64, N_EMBD], BF16, kind="ExternalInput")
    Y = nc.dram_tensor("y", [T, N_EMBD], BF16, kind="ExternalOutput")

    xt_ap = XT.ap()

    with tile.TileContext(nc) as tc:
        with (
            tc.tile_pool(name="const", bufs=1) as const_pool,
            tc.tile_pool(name="wpool", bufs=1) as wpool,
            tc.tile_pool(name="qkvt", bufs=1) as qkvt,
            tc.tile_pool(name="vsb", bufs=1) as vsb_pool,
            tc.tile_pool(name="ynt", bufs=1) as ynt_pool,
            tc.tile_pool(name="xs", bufs=3) as xs_pool,
            tc.tile_pool(name="ptp", bufs=6) as pt_pool,
            tc.tile_pool(name="ysb", bufs=3) as ysb_pool,
            tc.tile_pool(name="fin", bufs=3) as fin_pool,
            tc.tile_pool(name="rp", bufs=8) as r_pool,
            tc.tile_pool(name="yout", bufs=3) as yout_pool,
            tc.tile_pool(name="yqn", bufs=4) as yqn_pool,
            tc.tile_pool(name="pbig", bufs=2, space="PSUM") as pbig,
            tc.tile_pool(name="py", bufs=2, space="PSUM") as py_pool,
            tc.tile_pool(name="paux", bufs=2, space="PSUM") as paux,
        ):
            # ---- weights: direct bf16 DMA (first, so phase B isn't gated
            # on constant construction; spread across two idle queues) ----
            _weng = [nc.gpsimd, nc.scalar]

            def load_w(src_ap, shape, tag, i=[0]):
                t = wpool.tile(shape, BF16, tag=tag)
                _weng[i[0] % 2].dma_start(out=t, in_=src_ap)
                i[0] += 1
                return t

            wq01r = load_w(WQ01.ap().rearrange("(c p) m -> p c m", p=128), [128, CCHUNKS, 128], "wq01r")
            wk01r = load_w(WK01.ap().rearrange("(c p) m -> p c m", p=128), [128, CCHUNKS, 128], "wk01r")
            wv01r = load_w(WV01.ap().rearrange("(c p) m -> p c m", p=128), [128, CCHUNKS, 128], "wv01r")
            wqv2r = load_w(WQV2.ap().rearrange("(c p) m -> p c m", p=128), [128, CCHUNKS, 128], "wqv2r")
            wk2r = load_w(WK2.ap().rearrange("(c p) m -> p c m", p=128), [128, CCHUNKS, 64], "wk2r")
            wp1r = load_w(WP1.ap(), [128, N_EMBD], "wp1r")
            wp2r = load_w(WP2.ap(), [64, N_EMBD], "wp2r")

            # ---- constants ----
            ident_f = const_pool.tile([128, 128], F32)
            make_identity(nc, ident_f)
            identb = const_pool.tile([128, 128], BF16)
            nc.vector.tensor_copy(out=identb, in_=ident_f)
            bias_t = const_pool.tile([128, 1], F32)
            nc.vector.memset(bias_t, EXP_BIAS)

            # ---- persistent activations ----
            QT01 = qkvt.tile([128, T], BF16, tag="qt01")
            KT01 = qkvt.tile([128, T], BF16, tag="kt01")
            VT01 = qkvt.tile([128, T], BF16, tag="vt01")
            QV2 = qkvt.tile([128, T], BF16, tag="qv2")   # q_h2 rows 0:64, v_h2 rows 64:128
            KT2 = qkvt.tile([64, T], BF16, tag="kt2")
            # V in fp8, paired per DoubleRow k-tile: [keys, kt-pair, head,
            # j(2), 80] -- col 64 is the ones column (softmax denominator);
            # the 80-wide inner dim keeps the j-stride 16B-aligned as the
            # DoubleRow ldweights interleave requires.
            Vsb = vsb_pool.tile([128, n_kt // 2, HEADS_PER_CORE, 2, 80], FP8)
            # bf16 V copy for chunk 0 only: superblock 0's rows see few keys,
            # so fp8 P/V noise doesn't average out there -- those rows
            # (entirely contained in kt 0..3) take a bf16 PV path instead.
            Vsb0 = vsb_pool.tile([128, kt_per_qsb, HEADS_PER_CORE, 65], BF16,
                                 tag="vsb0")
            YnT01 = ynt_pool.tile([128, T], BF16, tag="ynt01")
            YnT2 = ynt_pool.tile([64, T], BF16, tag="ynt2")

            nc.vector.memset(
                Vsb[:, :, :, :, 64:65].rearrange("p a b c d -> p (a b c d)"),
                1.0)
            nc.vector.memset(
                Vsb0[:, :, :, 64:65].rearrange("p a b c -> p (a b c)"), 1.0)

            if phases != ('B', 'B2', 'C', 'D'):
                # timing-ablation build: zero every cross-phase tensor once so
                # skipped producers leave consumers with defined data
                for t in (QT01, KT01, VT01, QV2, YnT01):
                    nc.vector.memset(t, 0.0)
                for t in (KT2, YnT2):
                    nc.vector.memset(t, 0.0)
                nc.vector.memset(
                    Vsb[:, :, :, :, 0:64].rearrange("p a b c d -> p (a b c) d"),
                    0.125)
                nc.vector.memset(
                    Vsb0[:, :, :, 0:64].rearrange("p a b c -> p (a b) c"), 0.125)

            qkv_jobs = [
                (wq01r, QT01, 128), (wk01r, KT01, 128), (wv01r, VT01, 128),
                (wqv2r, QV2, 128), (wk2r, KT2, 64),
            ]

            for _ in range(repeat):
                # ---------- work generators ----------
                def b_chunk_gen(ch, split_dma=False):
                    """qkv projection for token chunk ch ([512] tokens)."""
                    sl = slice(ch * TOK_CHUNK, (ch + 1) * TOK_CHUNK)
                    xs = xs_pool.tile([128, CCHUNKS, TOK_CHUNK], BF16)
                    if split_dma:
                        # halve time-to-first-matmul at program start
                        h = CCHUNKS // 2
                        nc.sync.dma_start(out=xs[:, 0:h], in_=xt_ap[:, ch, 0:h])
                        nc.sync.dma_start(out=xs[:, h:], in_=xt_ap[:, ch, h:])
                    else:
                        nc.sync.dma_start(out=xs, in_=xt_ap[:, ch])
                    yield
                    for wt, out_sb, m in qkv_jobs:
                        ps = paux.tile([128, TOK_CHUNK], F32, tag="aux")
                        for c in range(CCHUNKS):
                            nc.tensor.matmul(
                                ps[0:m, :], wt[:, c, 0:m], xs[:, c, :],
                                start=(c == 0), stop=(c == CCHUNKS - 1),
                            )
                        nc.vector.tensor_copy(out=out_sb[0:m, sl], in_=ps[0:m, :])
                        yield

                def b2_gen(ch):
                    """V^T -> V (keys-major) transposes for chunk ch's key
                    tiles.  Heads 0+1 ride one [128,128] transpose."""
                    for kt in range(ch * kt_per_qsb, (ch + 1) * kt_per_qsb):
                        ks = slice(kt * KT, (kt + 1) * KT)
                        pv = paux.tile([128, 128], BF16, tag="aux")
                        nc.tensor.transpose(pv, VT01[:, ks], identb)
                        nc.vector.tensor_copy(
                            out=Vsb[:, kt // 2, 0:2, kt % 2, 0:64],
                            in_=pv.rearrange("p (b c) -> p b c", b=2))
                        if ch == 0:
                            nc.vector.tensor_copy(
                                out=Vsb0[:, kt, 0:2, 0:64],
                                in_=pv.rearrange("p (b c) -> p b c", b=2))
                        yield
                        pv2 = paux.tile([128, 64], BF16, tag="aux")
                        nc.tensor.transpose(pv2, QV2[64:128, ks], identb[64:128, 64:128])
                        nc.vector.tensor_copy(out=Vsb[:, kt // 2, 2, kt % 2, 0:64], in_=pv2)
                        if ch == 0:
                            nc.vector.tensor_copy(out=Vsb0[:, kt, 2, 0:64], in_=pv2)
                        yield

                def d_gen(qs):
                    """partial output projection for query superblock qs.
                    All four 128-token tiles stage into one buffer so the
                    write-back is a single [128, 4, 768] DMA."""
                    n_tt = QSB // 128
                    yo = yout_pool.tile([128, n_tt, N_EMBD], BF16)
                    for tt4 in range(n_tt):
                        tt = qs * n_tt + tt4
                        tsl = slice(tt * 128, (tt + 1) * 128)
                        for c0, ncols in ((0, 512), (512, 256)):
                            pp = paux.tile([128, 512], F32, tag="aux")
                            nc.tensor.matmul(pp[:, 0:ncols], YnT01[:, tsl],
                                             wp1r[:, c0:c0 + ncols], start=True, stop=False)
                            nc.tensor.matmul(pp[:, 0:ncols], YnT2[0:64, tsl],
                                             wp2r[0:64, c0:c0 + ncols], start=False, stop=True)
                            nc.vector.tensor_copy(out=yo[:, tt4, c0:c0 + ncols],
                                                  in_=pp[:, 0:ncols])
                            yield
                    nc.sync.dma_start(
                        out=Y.ap()[qs * QSB:(qs + 1) * QSB, :]
                            .rearrange("(tt p) c -> p tt c", p=128),
                        in_=yo)
                    yield

                # ---------- attention ----------
                head_qk = [
                    (QT01[0:64, :], KT01[0:64, :]),
                    (QT01[64:128, :], KT01[64:128, :]),
                    (QV2[0:64, :], KT2[0:64, :]),
                ]

                def attend_kloop_gen0(h, yps):
                    """superblock 0: bf16 per-tile PV (low-context rows)."""
                    qt_h, kt_h = head_qk[h]
                    nkt_q = kt_per_qsb
                    for kt2 in range(0, nkt_q, 2):
                        yield
                        last = (kt2 == nkt_q - 2)
                        q0 = QSB // 2 if last else 0
                        deltas = [(kt2 + j) * KT for j in range(2)]
                        q0s = [max(q0, min(d, QSB)) for d in deltas]
                        sps2 = pbig.tile([128, 2, QSB], F32, tag="big")
                        for j in range(2):
                            kt = kt2 + j
                            jsl = slice(q0s[j], QSB)
                            nc.tensor.matmul(sps2[:, j, jsl],
                                             kt_h[:, kt * KT:(kt + 1) * KT],
                                             qt_h[:, q0s[j]:QSB],
                                             start=True, stop=True)
                        pt2 = pt_pool.tile([128, 2, QSB], BF16, tag="pt0")
                        for j in range(2):
                            jsl = slice(q0s[j], QSB)
                            nc.scalar.activation(
                                out=pt2[:, j, jsl], in_=sps2[:, j, jsl],
                                func=mybir.ActivationFunctionType.Exp,
                                scale=float(HEAD_DIM) ** -0.5, bias=bias_t,
                            )
                            nc.gpsimd.affine_select(
                                out=pt2[:, j, jsl], in_=pt2[:, j, jsl],
                                compare_op=mybir.AluOpType.is_ge,
                                fill=0.0, base=q0s[j] - deltas[j],
                                channel_multiplier=-1,
                                pattern=[[1, QSB - q0s[j]]],
                            )
                        for j in range(2):
                            kt = kt2 + j
                            jsl = slice(q0s[j], QSB)
                            nc.tensor.matmul(yps[:, jsl], Vsb0[:, kt, h, :],
                                             pt2[:, j, jsl],
                                             start=(kt == 0),
                                             stop=(kt == nkt_q - 1))

                def attend_kloop_gen(h, qs, nkt_q, yps):
                    if qs == 0:
                        yield from attend_kloop_gen0(h, yps)
                        return
                    qt_h, kt_h = head_qk[h]
                    # PV for pair k is emitted at the START of segment k+1
                    # (and the last after the loop): the in-order PE queue
                    # then never sits behind a PV waiting on exp/select.
                    pend = None

                    def flush():
                        nonlocal pend
                        if pend is not None:
                            p_pt2, p_psl, p_kt2, p_last = pend
                            nc.tensor.matmul(
                                yps[:, p_psl], Vsb[:, p_kt2 // 2, h, :, 0:65],
                                p_pt2[:, :, p_psl],
                                start=(p_kt2 == 0), stop=p_last,
                                perf_mode=mybir.MatmulPerfMode.DoubleRow)
                            pend = None

                    for kt2 in range(0, nkt_q, 2):
                        yield
                        flush()
                        # diagonal tiles (delta > 0): query columns < delta
                        # see none of the tile's keys, so S runs ragged from
                        # column max(q0, delta).  The pair's PV is one fp8
                        # DoubleRow matmul over [q0p:QSB]; stale pt2 columns
                        # in [q0p:q0s[j]] are zeroed by the widened
                        # affine_select so they contribute nothing.
                        last = (kt2 == nkt_q - 2)
                        q0 = QSB // 2 if last else 0
                        deltas = [(kt2 + j) * KT - qs * QSB for j in range(2)]
                        q0s = [max(q0, min(d, QSB)) for d in deltas]
                        q0p = min(q0s)
                        sps2 = pbig.tile([128, 2, QSB], F32, tag="big")
                        for j in range(2):
                            kt = kt2 + j
                            ksl = slice(kt * KT, (kt + 1) * KT)
                            jsl = slice(q0s[j], QSB)
                            nc.tensor.matmul(sps2[:, j, jsl], kt_h[:, ksl],
                                             qt_h[:, qs * QSB + q0s[j]:(qs + 1) * QSB],
                                             start=True, stop=True)
                        pt2 = pt_pool.tile([128, 2, QSB], FP8)
                        if q0s[0] == q0s[1]:
                            nc.scalar.activation(
                                out=pt2[:, :, q0p:QSB], in_=sps2[:, :, q0p:QSB],
                                func=mybir.ActivationFunctionType.Exp,
                                scale=float(HEAD_DIM) ** -0.5, bias=bias_t,
                            )
                        else:
                            for j in range(2):
                                jsl = slice(q0s[j], QSB)
                                nc.scalar.activation(
                                    out=pt2[:, j, jsl], in_=sps2[:, j, jsl],
                                    func=mybir.ActivationFunctionType.Exp,
                                    scale=float(HEAD_DIM) ** -0.5, bias=bias_t,
                                )
                        for j in range(2):
                            delta = deltas[j]
                            if delta >= -KT + 1:
                                # keep P[i, idx] iff (q0p+idx) - i - delta >= 0
                                nc.gpsimd.affine_select(
                                    out=pt2[:, j, q0p:QSB], in_=pt2[:, j, q0p:QSB],
                                    compare_op=mybir.AluOpType.is_ge,
                                    fill=0.0, base=q0p - delta,
                                    channel_multiplier=-1,
                                    pattern=[[1, QSB - q0p]],
                                )
                        pend = (pt2, slice(q0p, QSB), kt2, last)
                    flush()

                def finish_gen(h, qs, yps):
                    """transpose + normalize Y^T for (h, qs).  All four
                    token tiles stage (transposed) into SBUF first so one
                    reciprocal serves the whole superblock."""
                    n_qt = QSB // 128
                    ysb = ysb_pool.tile([65, QSB], BF16)
                    nc.vector.tensor_copy(out=ysb, in_=yps)
                    yield
                    st = fin_pool.tile([128, n_qt, 65], BF16)
                    for qt in range(n_qt):
                        pt1 = paux.tile([128, 65], BF16, tag="aux")
                        nc.tensor.transpose(
                            pt1, ysb[:, qt * 128:(qt + 1) * 128], identb[0:65, 0:65])
                        nc.vector.tensor_copy(out=st[:, qt, :], in_=pt1)
                        if qt < n_qt - 1:
                            yield
                    rr = r_pool.tile([128, n_qt], F32)
                    nc.vector.reciprocal(rr, st[:, :, 64])
                    yield
                    for qt in range(n_qt):
                        csl = slice(qs * QSB + qt * 128, qs * QSB + (qt + 1) * 128)
                        yqn = yqn_pool.tile([128, 64], BF16)
                        nc.vector.tensor_scalar_mul(yqn, st[:, qt, 0:64],
                                                    rr[:, qt:qt + 1])
                        pt2r = paux.tile([64, 128], BF16, tag="aux")
                        nc.tensor.transpose(pt2r, yqn, identb)
                        if h == 0:
                            dst = YnT01[0:64, csl]
                        elif h == 1:
                            dst = YnT01[64:128, csl]
                        else:
                            dst = YnT2[0:64, csl]
                        nc.vector.tensor_copy(out=dst, in_=pt2r)
                        yield

                # ---------- interleaved schedule ----------
                side = deque()     # FIFO of generators (b/b2/d work)
                bwork = {}         # ch -> [gens] that must be emitted before
                                   # attention touches chunk ch

                def pull(n=1):
                    for _ in range(n):
                        while side:
                            try:
                                next(side[0])
                                break
                            except StopIteration:
                                side.popleft()
                        else:
                            return

                def drain(gens):
                    for g in gens:
                        for _ in g:
                            pass

                def drain_bwork_through(ch):
                    for c in range(ch + 1):
                        for g in bwork.pop(c, ()):
                            # may already be partially consumed via `side`
                            for _ in g:
                                pass

                def rotate(gens, pulls=1):
                    live = list(gens)
                    while live:
                        nxt = []
                        for g in live:
                            try:
                                next(g)
                                nxt.append(g)
                            except StopIteration:
                                pass
                            pull(pulls)
                        live = nxt

                has = lambda p: p in phases
                # prologue: chunk 0 must be ready before attention qs=0
                if has('B'):
                    drain([b_chunk_gen(0, split_dma=True)])
                if has('B2'):
                    drain([b2_gen(0)])

                if not has('C'):
                    for ch in range(1, n_qsb):
                        if has('B'):
                            drain([b_chunk_gen(ch)])
                        if has('B2'):
                            drain([b2_gen(ch)])
                    if has('D'):
                        for qs in range(n_qsb):
                            drain([d_gen(qs)])
                    continue

                fin2_prev = None   # finish gen of head 2 from previous qs
                for qs in range(n_qsb):
                    if qs + 1 < n_qsb:
                        gens = ([b_chunk_gen(qs + 1)] if has('B') else []) + \
                               ([b2_gen(qs + 1)] if has('B2') else [])
                        bwork[qs + 1] = gens
                        side.extend(gens)
                    # attention qs needs chunks <= qs fully emitted
                    drain_bwork_through(qs)

                    nkt_q = (qs + 1) * kt_per_qsb
                    yps0 = py_pool.tile([65, QSB], F32, tag="y", name=f"yps0_{qs}")
                    yps1 = py_pool.tile([65, QSB], F32, tag="y", name=f"yps1_{qs}")
                    g0 = attend_kloop_gen(0, qs, nkt_q, yps0)
                    g1 = attend_kloop_gen(1, qs, nkt_q, yps1)
                    rot_a = ([fin2_prev] if fin2_prev is not None else []) + [g0, g1]
                    rotate(rot_a, pulls=1)

                    if qs >= 1 and has('D'):
                        side.append(d_gen(qs - 1))
                    yps2 = py_pool.tile([65, QSB], F32, tag="y", name=f"yps2_{qs}")
                    g2 = attend_kloop_gen(2, qs, nkt_q, yps2)
                    f0 = finish_gen(0, qs, yps0)
                    f1 = finish_gen(1, qs, yps1)
                    rotate([f0, f1, g2], pulls=2)
                    fin2_prev = finish_gen(2, qs, yps2)

                # epilogue: lockstep head-2's last finish with the last
                # projection block (d tt-k needs fin2's qt-k written first)
                if not has('D'):
                    drain([fin2_prev])
                    pull(10 ** 9)
                    continue
                f, dg = fin2_prev, d_gen(n_qsb - 1)
                for _ in range(6):   # ysb, 3 staging transposes, recip, YnT qt0
                    next(f)
                for _k in range(QSB // 128):
                    next(dg)              # tt-k first half (reads qt-k cols)
                    try:
                        next(f)           # qt-(k+1)
                    except StopIteration:
                        pass
                    next(dg)              # tt-k second half
                next(dg)                  # batched y DMA
                pull(10 ** 9)

    nc.compile()
    return nc


def make_in_maps(x, w_qkv, w_proj, T=T_FULL):
    """Per-core input dicts from full inputs (numpy), bf16-converted."""
    x = np.asarray(x, dtype=np.float32)
    w_qkv = np.asarray(w_qkv, dtype=np.float32).astype(NP_BF16)
    w_proj = np.asarray(w_proj, dtype=np.float32).astype(NP_BF16)
    cores_per_batch = N_CORES // B
    # x^T swizzled to [p, chunk, cchunk, tok] so each chunk DMA is one
    # contiguous run per partition (see XT in build_nc)
    n_ch = T // TOK_CHUNK
    xt_b = []
    for b in range(B):
        xt = x[b].T.reshape(CCHUNKS, 128, n_ch, TOK_CHUNK)
        xt_b.append(np.ascontiguousarray(
            xt.transpose(1, 2, 0, 3)).astype(NP_BF16))
    in_maps = []
    for core in range(N_CORES):
        b = core // cores_per_batch
        h0 = (core % cores_per_batch) * HEADS_PER_CORE
        h1, h2 = h0 + 1, h0 + 2
        col = lambda kind, h: w_qkv[:, kind * N_EMBD + h * HEAD_DIM:
                                    kind * N_EMBD + (h + 1) * HEAD_DIM]
        in_maps.append({
            "xt": xt_b[b],
            "wq01": np.ascontiguousarray(np.concatenate([col(0, h0), col(0, h1)], axis=1)),
            "wk01": np.ascontiguousarray(np.concatenate([col(1, h0), col(1, h1)], axis=1)),
            "wv01": np.ascontiguousarray(np.concatenate([col(2, h0), col(2, h1)], axis=1)),
            "wqv2": np.ascontiguousarray(np.concatenate([col(0, h2), col(2, h2)], axis=1)),
            "wk2": np.ascontiguousarray(col(1, h2)),
            "wp1": np.ascontiguousarray(w_proj[h0 * HEAD_DIM:(h1 + 1) * HEAD_DIM, :]),
            "wp2": np.ascontiguousarray(w_proj[h2 * HEAD_DIM:(h2 + 1) * HEAD_DIM, :]),
        })
    return in_maps


def gather_output(results, T=T_FULL):
    cores_per_batch = N_CORES // B
    out = np.empty((B, T, N_EMBD), dtype=np.float32)
    for b in range(B):
        parts = [np.asarray(results[b * cores_per_batch + j]["y"], dtype=np.float32)
                 for j in range(cores_per_batch)]
        out[b] = parts[0] + parts[1] + parts[2] + parts[3]
    return out


_CACHE = {}


def _get_nc(T=T_FULL, repeat=1):
    key = (T, repeat)
    if key not in _CACHE:
        _CACHE[key] = build_nc(T, repeat)
    return _CACHE[key]


def kernel(x, w_qkv, w_proj):
    import time as _time
    from concourse.bass_utils import run_bass_kernel_spmd
    T = x.shape[1]
    nc = _get_nc(T)
    in_maps = make_in_maps(x, w_qkv, w_proj, T)
    last_err = None
    for attempt in range(3):
        try:
            res = run_bass_kernel_spmd(nc, in_maps, list(range(N_CORES)))
            return gather_output(res.results, T)
        except Exception as e:  # transient device wedge: retry after a pause
            last_err = e
            _time.sleep(20 * (attempt + 1))
    raise last_err



# revision 33
# speedup vs baseline: 1.6781x; 1.6781x over previous
"""Causal self-attention (B=2, T=4096, C=768, H=12) on 8 trn2 NeuronCores.

Sharding: data-parallel on batch (cores 0-3 -> batch 0, cores 4-7 -> batch 1),
tensor-parallel on heads (3 heads per core).  Each core computes qkv for its
3 heads, causal flash-style attention, and a partial output projection
(its heads' rows of w_proj); the host sums the 4 partials per batch.

v11 structure (vs the serial-phase v7 baseline, ~2.05x faster: 676us ->
330us per iteration measured via repeat-differencing with block sampling):
- All activations/weights in bf16 (host-converted): halves DMA traffic and
  removes every fp32->fp32r rounding copy.  PSUM accumulation stays fp32.
  Partial Y outputs are written bf16 and summed fp32 on the host.
- Causal masking via gpsimd affine_select directly on the exp'd P tile
  (Pool engine is otherwise idle), freeing DVE; diagonal tiles compute
  S/PV ragged (columns left of the diagonal tile are skipped).
- One software-pipelined loop: the qkv projection chunk qs+1, V^T->V
  transposes, and the output projection for qs-1 are emitted interleaved
  into the attention rotation for query superblock qs, so their DMA/PE/
  DVE work hides under the attention inner loop (PE ~83% busy in sim).
- x^T is host-swizzled to [partition, chunk, cchunk, token] so each chunk
  DMA is one contiguous 6KB run per partition; y writes batch 4 token
  tiles per DMA.
"""

import sys

if '/opt/trn_rl_repo' not in sys.path:
    sys.path.insert(0, '/opt/trn_rl_repo')

from collections import deque

import numpy as np
import ml_dtypes

import concourse.bacc as bacc
import concourse.mybir as mybir
import concourse.tile as tile
from concourse.masks import make_identity

dt = mybir.dt
F32 = dt.float32
BF16 = dt.bfloat16
FP8 = dt.float8e4
NP_BF16 = ml_dtypes.bfloat16

# exp bias (in log space) applied to every attention logit before the fp8
# P tile: keeps exp(max_logit)+margin under the TRN fp8e4 max of 240 while
# keeping every row's max P far above the subnormal flush threshold.  The
# uniform scale cancels in the softmax normalization.
EXP_BIAS = -3.0 * float(np.log(2.0))

N_EMBD = 768
N_HEADS = 12
HEAD_DIM = 64
B = 2
T_FULL = 4096
N_CORES = 8
HEADS_PER_CORE = N_HEADS // (N_CORES // B)  # 3

TOK_CHUNK = 512   # qkv phase token chunk == query superblock
QSB = 512         # attention query superblock
KT = 128          # key tile (contraction for P@V)
CCHUNKS = N_EMBD // 128  # 6 contraction chunks


def build_nc(T=T_FULL, repeat=1, phases=('B', 'B2', 'C', 'D')):
    """Build the per-core Bass program.  Same program runs SPMD on all 8
    cores; per-core data (x^T of its batch, its heads' weight slices) comes
    via the input map.  `phases` subsets the per-iteration work (timing
    ablation only -- outputs are garbage unless all phases run)."""
    nc = bacc.Bacc(None, target_bir_lowering=False, debug=False)

    n_kt = T // KT
    n_qsb = T // QSB
    n_tok = T // 128
    kt_per_qsb = QSB // KT  # 4

    # x^T pre-swizzled on host to [p, chunk, cchunk, tok]: each chunk DMA
    # reads one contiguous 6KB run per partition.
    XT = nc.dram_tensor(
        "xt", [128, T // TOK_CHUNK, CCHUNKS, TOK_CHUNK], BF16,
        kind="ExternalInput")
    WQ01 = nc.dram_tensor("wq01", [N_EMBD, 128], BF16, kind="ExternalInput")
    WK01 = nc.dram_tensor("wk01", [N_EMBD, 128], BF16, kind="ExternalInput")
    WV01 = nc.dram_tensor("wv01", [N_EMBD, 128], BF16, kind="ExternalInput")
    WQV2 = nc.dram_tensor("wqv2", [N_EMBD, 128], BF16, kind="ExternalInput")
    WK2 = nc.dram_tensor("wk2", [N_EMBD, 64], BF16, kind="ExternalInput")
    WP1 = nc.dram_tensor("wp1", [128, N_EMBD], BF16, kind="ExternalInput")
    WP2 = nc.dram_tensor("wp2", [64, N_EMBD], BF16, kind="ExternalInput")
    Y = nc.dram_tensor("y", [T, N_EMBD], BF16, kind="ExternalOutput")

    xt_ap = XT.ap()

    with tile.TileContext(nc) as tc:
        with (
            tc.tile_pool(name="const", bufs=1) as const_pool,
            tc.tile_pool(name="wpool", bufs=1) as wpool,
            tc.tile_pool(name="qkvt", bufs=1) as qkvt,
            tc.tile_pool(name="vsb", bufs=1) as vsb_pool,
            tc.tile_pool(name="ynt", bufs=1) as ynt_pool,
            tc.tile_pool(name="xs", bufs=3) as xs_pool,
            tc.tile_pool(name="ptp", bufs=6) as pt_pool,
            tc.tile_pool(name="ysb", bufs=3) as ysb_pool,
            tc.tile_pool(name="fin", bufs=3) as fin_pool,
            tc.tile_pool(name="rp", bufs=8) as r_pool,
            tc.tile_pool(name="yout", bufs=3) as yout_pool,
            tc.tile_pool(name="yqn", bufs=4) as yqn_pool,
            tc.tile_pool(name="pbig", bufs=2, space="PSUM") as pbig,
            tc.tile_pool(name="py", bufs=2, space="PSUM") as py_pool,
            tc.tile_pool(name="paux", bufs=2, space="PSUM") as paux,
        ):
            # ---- weights: direct bf16 DMA (first, so phase B isn't gated
            # on constant construction; spread across two idle queues) ----
            _weng = [nc.gpsimd, nc.scalar]

            def load_w(src_ap, shape, tag, i=[0]):
                t = wpool.tile(shape, BF16, tag=tag)
                _weng[i[0] % 2].dma_start(out=t, in_=src_ap)
                i[0] += 1
                return t

            wq01r = load_w(WQ01.ap().rearrange("(c p) m -> p c m", p=128), [128, CCHUNKS, 128], "wq01r")
            wk01r = load_w(WK01.ap().rearrange("(c p) m -> p c m", p=128), [128, CCHUNKS, 128], "wk01r")
            wv01r = load_w(WV01.ap().rearrange("(c p) m -> p c m", p=128), [128, CCHUNKS, 128], "wv01r")
            wqv2r = load_w(WQV2.ap().rearrange("(c p) m -> p c m", p=128), [128, CCHUNKS, 128], "wqv2r")
            wk2r = load_w(WK2.ap().rearrange("(c p) m -> p c m", p=128), [128, CCHUNKS, 64], "wk2r")
            wp1r = load_w(WP1.ap(), [128, N_EMBD], "wp1r")
            wp2r = load_w(WP2.ap(), [64, N_EMBD], "wp2r")

            # ---- constants ----
            ident_f = const_pool.tile([128, 128], F32)
            make_identity(nc, ident_f)
            identb = const_pool.tile([128, 128], BF16)
            nc.vector.tensor_copy(out=identb, in_=ident_f)
            bias_t = const_pool.tile([128, 1], F32)
            nc.vector.memset(bias_t, EXP_BIAS)

            # ---- persistent activations ----
            QT01 = qkvt.tile([128, T], BF16, tag="qt01")
            KT01 = qkvt.tile([128, T], BF16, tag="kt01")
            VT01 = qkvt.tile([128, T], BF16, tag="vt01")
            QV2 = qkvt.tile([128, T], BF16, tag="qv2")   # q_h2 rows 0:64, v_h2 rows 64:128
            KT2 = qkvt.tile([64, T], BF16, tag="kt2")
            # V in fp8, paired per DoubleRow k-tile: [keys, kt-pair, head,
            # j(2), 80] -- col 64 is the ones column (softmax denominator);
            # the 80-wide inner dim keeps the j-stride 16B-aligned as the
            # DoubleRow ldweights interleave requires.
            Vsb = vsb_pool.tile([128, n_kt // 2, HEADS_PER_CORE, 2, 80], FP8)
            # bf16 V copy for chunk 0 only: superblock 0's rows see few keys,
            # so fp8 P/V noise doesn't average out there -- those rows
            # (entirely contained in kt 0..3) take a bf16 PV path instead.
            Vsb0 = vsb_pool.tile([128, kt_per_qsb, HEADS_PER_CORE, 65], BF16,
                                 tag="vsb0")
            YnT01 = ynt_pool.tile([128, T], BF16, tag="ynt01")
            YnT2 = ynt_pool.tile([64, T], BF16, tag="ynt2")

            nc.vector.memset(
                Vsb[:, :, :, :, 64:65].rearrange("p a b c d -> p (a b c d)"),
                1.0)
            nc.vector.memset(
                Vsb0[:, :, :, 64:65].rearrange("p a b c -> p (a b c)"), 1.0)

            if phases != ('B', 'B2', 'C', 'D'):
                # timing-ablation build: zero every cross-phase tensor once so
                # skipped producers leave consumers with defined data
                for t in (QT01, KT01, VT01, QV2, YnT01):
                    nc.vector.memset(t, 0.0)
                for t in (KT2, YnT2):
                    nc.vector.memset(t, 0.0)
                nc.vector.memset(
                    Vsb[:, :, :, :, 0:64].rearrange("p a b c d -> p (a b c) d"),
                    0.125)
                nc.vector.memset(
                    Vsb0[:, :, :, 0:64].rearrange("p a b c -> p (a b) c"), 0.125)

            qkv_jobs = [
                (wq01r, QT01, 128), (wk01r, KT01, 128), (wv01r, VT01, 128),
                (wqv2r, QV2, 128), (wk2r, KT2, 64),
            ]

            for _ in range(repeat):
                # ---------- work generators ----------
                def b_chunk_gen(ch, split_dma=False):
                    """qkv projection for token chunk ch ([512] tokens).
                    Yields every 2 contraction chunks so pulled side work
                    stays fine-grained (~1k PE cycles per step)."""
                    sl = slice(ch * TOK_CHUNK, (ch + 1) * TOK_CHUNK)
                    xs = xs_pool.tile([128, CCHUNKS, TOK_CHUNK], BF16)
                    if split_dma:
                        # halve time-to-first-matmul at program start
                        h = CCHUNKS // 2
                        nc.sync.dma_start(out=xs[:, 0:h], in_=xt_ap[:, ch, 0:h])
                        nc.sync.dma_start(out=xs[:, h:], in_=xt_ap[:, ch, h:])
                    else:
                        nc.sync.dma_start(out=xs, in_=xt_ap[:, ch])
                    yield
                    for wt, out_sb, m in qkv_jobs:
                        ps = paux.tile([128, TOK_CHUNK], F32, tag="aux")
                        for c in range(CCHUNKS):
                            nc.tensor.matmul(
                                ps[0:m, :], wt[:, c, 0:m], xs[:, c, :],
                                start=(c == 0), stop=(c == CCHUNKS - 1),
                            )
                            if c != CCHUNKS - 1:
                                yield
                        nc.vector.tensor_copy(out=out_sb[0:m, sl], in_=ps[0:m, :])
                        yield

                def b2_gen(ch):
                    """V^T -> V (keys-major) transposes for chunk ch's key
                    tiles.  Heads 0+1 ride one [128,128] transpose."""
                    for kt in range(ch * kt_per_qsb, (ch + 1) * kt_per_qsb):
                        ks = slice(kt * KT, (kt + 1) * KT)
                        pv = paux.tile([128, 128], BF16, tag="aux")
                        nc.tensor.transpose(pv, VT01[:, ks], identb)
                        nc.vector.tensor_copy(
                            out=Vsb[:, kt // 2, 0:2, kt % 2, 0:64],
                            in_=pv.rearrange("p (b c) -> p b c", b=2))
                        if ch == 0:
                            nc.vector.tensor_copy(
                                out=Vsb0[:, kt, 0:2, 0:64],
                                in_=pv.rearrange("p (b c) -> p b c", b=2))
                        yield
                        pv2 = paux.tile([128, 64], BF16, tag="aux")
                        nc.tensor.transpose(pv2, QV2[64:128, ks], identb[64:128, 64:128])
                        nc.vector.tensor_copy(out=Vsb[:, kt // 2, 2, kt % 2, 0:64], in_=pv2)
                        if ch == 0:
                            nc.vector.tensor_copy(out=Vsb0[:, kt, 2, 0:64], in_=pv2)
                        yield

                def d_gen(qs, t0=0, t1=QSB // 128):
                    """partial output projection for superblock qs token
                    tiles [t0, t1).  Tiles stage into one buffer; for the
                    full window the write-back goes out in two DMAs so the
                    first overlaps the second half's matmuls."""
                    n_tt = QSB // 128
                    full = (t0 == 0 and t1 == n_tt)
                    yo = yout_pool.tile([128, n_tt, N_EMBD], BF16)
                    for tt4 in range(t0, t1):
                        tt = qs * n_tt + tt4
                        tsl = slice(tt * 128, (tt + 1) * 128)
                        for c0, ncols in ((0, 512), (512, 256)):
                            pp = paux.tile([128, 512], F32, tag="aux")
                            nc.tensor.matmul(pp[:, 0:ncols], YnT01[:, tsl],
                                             wp1r[:, c0:c0 + ncols], start=True, stop=False)
                            nc.tensor.matmul(pp[:, 0:ncols], YnT2[0:64, tsl],
                                             wp2r[0:64, c0:c0 + ncols], start=False, stop=True)
                            nc.vector.tensor_copy(out=yo[:, tt4, c0:c0 + ncols],
                                                  in_=pp[:, 0:ncols])
                            yield
                        if full and tt4 == n_tt // 2 - 1:
                            nc.sync.dma_start(
                                out=Y.ap()[qs * QSB:qs * QSB + QSB // 2, :]
                                    .rearrange("(tt p) c -> p tt c", p=128),
                                in_=yo[:, 0:n_tt // 2])
                    lo = qs * QSB + (QSB // 2 if full else t0 * 128)
                    nc.sync.dma_start(
                        out=Y.ap()[lo:qs * QSB + t1 * 128, :]
                            .rearrange("(tt p) c -> p tt c", p=128),
                        in_=yo[:, (n_tt // 2 if full else t0):t1])
                    yield

                # ---------- attention ----------
                head_qk = [
                    (QT01[0:64, :], KT01[0:64, :]),
                    (QT01[64:128, :], KT01[64:128, :]),
                    (QV2[0:64, :], KT2[0:64, :]),
                ]

                def attend_kloop_gen0(h, yps):
                    """superblock 0: bf16 per-tile PV (low-context rows)."""
                    qt_h, kt_h = head_qk[h]
                    nkt_q = kt_per_qsb
                    for kt2 in range(0, nkt_q, 2):
                        yield
                        last = (kt2 == nkt_q - 2)
                        q0 = QSB // 2 if last else 0
                        deltas = [(kt2 + j) * KT for j in range(2)]
                        q0s = [max(q0, min(d, QSB)) for d in deltas]
                        sps2 = pbig.tile([128, 2, QSB], F32, tag="big")
                        for j in range(2):
                            kt = kt2 + j
                            jsl = slice(q0s[j], QSB)
                            nc.tensor.matmul(sps2[:, j, jsl],
                                             kt_h[:, kt * KT:(kt + 1) * KT],
                                             qt_h[:, q0s[j]:QSB],
                                             start=True, stop=True)
                        pt2 = pt_pool.tile([128, 2, QSB], BF16, tag="pt0")
                        for j in range(2):
                            jsl = slice(q0s[j], QSB)
                            nc.scalar.activation(
                                out=pt2[:, j, jsl], in_=sps2[:, j, jsl],
                                func=mybir.ActivationFunctionType.Exp,
                                scale=float(HEAD_DIM) ** -0.5, bias=bias_t,
                            )
                            nc.gpsimd.affine_select(
                                out=pt2[:, j, jsl], in_=pt2[:, j, jsl],
                                compare_op=mybir.AluOpType.is_ge,
                                fill=0.0, base=q0s[j] - deltas[j],
                                channel_multiplier=-1,
                                pattern=[[1, QSB - q0s[j]]],
                            )
                        for j in range(2):
                            kt = kt2 + j
                            jsl = slice(q0s[j], QSB)
                            nc.tensor.matmul(yps[:, jsl], Vsb0[:, kt, h, :],
                                             pt2[:, j, jsl],
                                             start=(kt == 0),
                                             stop=(kt == nkt_q - 1))

                def attend_kloop_gen(h, qs, qw0, qw1, yps):
                    """causal attention of head h for queries
                    [qs*QSB+qw0, qs*QSB+qw1) accumulated into
                    yps[:, qw0:qw1]."""
                    if qs == 0:
                        yield from attend_kloop_gen0(h, yps)
                        return
                    qt_h, kt_h = head_qk[h]
                    nkt_w = (qs * QSB + qw1) // KT
                    # key tiles per exp group: keeps ~1024 elems/partition
                    # per activation instruction regardless of window width.
                    grp = 4 if qw1 - qw0 <= QSB // 2 else 2
                    assert nkt_w % 2 == 0

                    def s_group(kt2):
                        # diagonal tiles: query columns left of the tile's
                        # delta see none of its keys, so S runs ragged from
                        # max(qw0, delta).  Each PV pair is one fp8
                        # DoubleRow matmul over [q0p_pair:qw1]; stale pt2
                        # columns in [q0p:q0s[j]] are zeroed by the widened
                        # affine_select so they contribute nothing.
                        g = min(grp, nkt_w - kt2)
                        deltas = [(kt2 + j) * KT - qs * QSB for j in range(g)]
                        q0s = [max(qw0, min(d, qw1)) for d in deltas]
                        q0p = min(q0s)
                        sps = pbig.tile([128, g, qw1 - q0p], F32, tag="big")
                        for j in range(g):
                            kt = kt2 + j
                            ksl = slice(kt * KT, (kt + 1) * KT)
                            nc.tensor.matmul(sps[:, j, q0s[j] - q0p:], kt_h[:, ksl],
                                             qt_h[:, qs * QSB + q0s[j]:qs * QSB + qw1],
                                             start=True, stop=True)
                        return (kt2, g, sps, deltas, q0s, q0p)

                    # S is emitted one group AHEAD of its exp/PV segment so
                    # the in-order PE queue always has the next S in flight
                    # before it reaches PV_k (which waits on exp_k) -- keeps
                    # the ACT exp stream gapless.
                    nxt = None
                    for kt2 in range(0, nkt_w, grp):
                        yield
                        if nxt is None:
                            nxt = s_group(0)
                        cur = nxt
                        nxt = s_group(kt2 + grp) if kt2 + grp < nkt_w else None
                        _, g, sps, deltas, q0s, q0p = cur
                        pt2 = pt_pool.tile([128, g, qw1 - q0p], FP8)
                        if all(q == q0p for q in q0s):
                            nc.scalar.activation(
                                out=pt2, in_=sps,
                                func=mybir.ActivationFunctionType.Exp,
                                scale=float(HEAD_DIM) ** -0.5, bias=bias_t,
                            )
                        else:
                            for j in range(g):
                                jsl = slice(q0s[j] - q0p, qw1 - q0p)
                                nc.scalar.activation(
                                    out=pt2[:, j, jsl], in_=sps[:, j, jsl],
                                    func=mybir.ActivationFunctionType.Exp,
                                    scale=float(HEAD_DIM) ** -0.5, bias=bias_t,
                                )
                        for j in range(g):
                            delta = deltas[j]
                            if delta + KT - 1 > q0s[j] or q0s[j] > q0p:
                                # keep P[i, idx] iff (q0p+idx) - i - delta >= 0
                                nc.gpsimd.affine_select(
                                    out=pt2[:, j, :], in_=pt2[:, j, :],
                                    compare_op=mybir.AluOpType.is_ge,
                                    fill=0.0, base=q0p - delta,
                                    channel_multiplier=-1,
                                    pattern=[[1, qw1 - q0p]],
                                )
                        for jj in range(0, g, 2):
                            q0pp = min(q0s[jj], q0s[jj + 1])
                            nc.tensor.matmul(
                                yps[:, q0pp:qw1],
                                Vsb[:, (kt2 + jj) // 2, h, :, 0:65],
                                pt2[:, jj:jj + 2, q0pp - q0p:],
                                start=(kt2 + jj == 0),
                                stop=(kt2 + jj == nkt_w - 2),
                                perf_mode=mybir.MatmulPerfMode.DoubleRow)

                def finish_gen(h, qs, yps, t0=0, t1=QSB // 128):
                    """transpose + normalize Y^T for (h, qs) token tiles
                    [t0, t1).  Tiles stage (transposed) into SBUF first so
                    one reciprocal serves the window."""
                    n_w = t1 - t0
                    ysb = ysb_pool.tile([65, QSB], BF16)
                    nc.vector.tensor_copy(out=ysb[:, t0 * 128:t1 * 128],
                                          in_=yps[:, t0 * 128:t1 * 128])
                    yield
                    st = fin_pool.tile([128, QSB // 128, 65], BF16)
                    for qt in range(t0, t1):
                        pt1 = paux.tile([128, 65], BF16, tag="aux")
                        nc.tensor.transpose(
                            pt1, ysb[:, qt * 128:(qt + 1) * 128], identb[0:65, 0:65])
                        nc.vector.tensor_copy(out=st[:, qt, :], in_=pt1)
                        if qt < t1 - 1:
                            yield
                    rr = r_pool.tile([128, QSB // 128], F32)
                    nc.vector.reciprocal(rr[:, t0:t1], st[:, t0:t1, 64])
                    yield
                    for qt in range(t0, t1):
                        csl = slice(qs * QSB + qt * 128, qs * QSB + (qt + 1) * 128)
                        yqn = yqn_pool.tile([128, 64], BF16)
                        nc.vector.tensor_scalar_mul(yqn, st[:, qt, 0:64],
                                                    rr[:, qt:qt + 1])
                        pt2r = paux.tile([64, 128], BF16, tag="aux")
                        nc.tensor.transpose(pt2r, yqn, identb)
                        if h == 0:
                            dst = YnT01[0:64, csl]
                        elif h == 1:
                            dst = YnT01[64:128, csl]
                        else:
                            dst = YnT2[0:64, csl]
                        nc.vector.tensor_copy(out=dst, in_=pt2r)
                        yield

                # ---------- interleaved schedule ----------
                side = deque()     # FIFO of generators (b/b2/d work)
                bwork = {}         # ch -> [gens] that must be emitted before
                                   # attention touches chunk ch

                def pull(n=1):
                    for _ in range(n):
                        while side:
                            try:
                                next(side[0])
                                break
                            except StopIteration:
                                side.popleft()
                        else:
                            return

                def drain(gens):
                    for g in gens:
                        for _ in g:
                            pass

                def drain_bwork_through(ch):
                    for c in range(ch + 1):
                        for g in bwork.pop(c, ()):
                            # may already be partially consumed via `side`
                            for _ in g:
                                pass

                def rotate(gens):
                    """gens: list of (gen, pulls_after_each_step).  Pulls
                    ride attention steps (ACT-paced, PE slack) rather than
                    finish steps so the exp stream stays dense."""
                    live = list(gens)
                    while live:
                        nxt = []
                        for g, npull in live:
                            try:
                                next(g)
                                nxt.append((g, npull))
                            except StopIteration:
                                pass
                            pull(npull)
                        live = nxt

                has = lambda p: p in phases
                # prologue: chunk 0 must be ready before attention qs=0;
                # chunk 1 is enqueued right away so each b(ch) has two full
                # superblocks of pull capacity before its forced drain.
                if has('B'):
                    drain([b_chunk_gen(0, split_dma=True)])
                if has('B2'):
                    drain([b2_gen(0)])
                if n_qsb > 1:
                    gens1 = ([b_chunk_gen(1)] if has('B') else []) + \
                            ([b2_gen(1)] if has('B2') else [])
                    bwork[1] = gens1
                    side.extend(gens1)

                if not has('C'):
                    for ch in range(1, n_qsb):
                        if has('B'):
                            drain([b_chunk_gen(ch)])
                        if has('B2'):
                            drain([b2_gen(ch)])
                    if has('D'):
                        for qs in range(n_qsb):
                            drain([d_gen(qs)])
                    continue

                fin2_prev = None   # finish gen of head 2 from previous qs
                for qs in range(n_qsb):
                    if qs + 2 < n_qsb:
                        gens = ([b_chunk_gen(qs + 2)] if has('B') else []) + \
                               ([b2_gen(qs + 2)] if has('B2') else [])
                        bwork[qs + 2] = gens
                        side.extend(gens)
                    # attention qs needs chunks <= qs fully emitted
                    drain_bwork_through(qs)

                    split_tail = has('D') and qs == n_qsb - 1 and n_qsb > 1
                    yps0 = py_pool.tile([65, QSB], F32, tag="y", name=f"yps0_{qs}")
                    yps1 = py_pool.tile([65, QSB], F32, tag="y", name=f"yps1_{qs}")
                    # for the last superblock, attention runs in two query
                    # half-windows so window A's finish+projection overlap
                    # window B's attention instead of serializing after it.
                    qw1 = QSB // 2 if split_tail else QSB
                    ht = qw1 // 128
                    g0 = attend_kloop_gen(0, qs, 0, qw1, yps0)
                    g1 = attend_kloop_gen(1, qs, 0, qw1, yps1)
                    rot_a = ([(fin2_prev, 1)] if fin2_prev is not None else []) \
                        + [(g0, 1), (g1, 1)]
                    rotate(rot_a)

                    if qs >= 1 and has('D'):
                        side.append(d_gen(qs - 1))
                    yps2 = py_pool.tile([65, QSB], F32, tag="y", name=f"yps2_{qs}")
                    g2 = attend_kloop_gen(2, qs, 0, qw1, yps2)
                    f0 = finish_gen(0, qs, yps0, 0, ht)
                    f1 = finish_gen(1, qs, yps1, 0, ht)
                    rotate([(f0, 0), (f1, 0), (g2, 1)])
                    fin2_prev = finish_gen(2, qs, yps2, 0, ht)

                    if split_tail:
                        # window B gets fresh yps tiles: a PSUM tile supports
                        # only one accumulation generation (one start=True
                        # group) in the tile framework.
                        yps0b = py_pool.tile([65, QSB], F32, tag="y", name="yps0b")
                        yps1b = py_pool.tile([65, QSB], F32, tag="y", name="yps1b")
                        g0b = attend_kloop_gen(0, qs, qw1, QSB, yps0b)
                        g1b = attend_kloop_gen(1, qs, qw1, QSB, yps1b)
                        rotate([(fin2_prev, 1), (g0b, 1), (g1b, 1)])
                        side.append(d_gen(qs, 0, ht))   # project window A
                        yps2b = py_pool.tile([65, QSB], F32, tag="y", name="yps2b")
                        g2b = attend_kloop_gen(2, qs, qw1, QSB, yps2b)
                        f0b = finish_gen(0, qs, yps0b, ht, QSB // 128)
                        f1b = finish_gen(1, qs, yps1b, ht, QSB // 128)
                        rotate([(f0b, 0), (f1b, 0), (g2b, 1)])
                        fin2_prev = finish_gen(2, qs, yps2b, ht, QSB // 128)

                # epilogue: lockstep head-2's last finish with the last
                # projection window (d tt-k needs fin2's qt-k written first)
                if not has('D'):
                    drain([fin2_prev])
                    pull(10 ** 9)
                    continue
                n_tt = QSB // 128
                t0 = n_tt // 2 if n_qsb > 1 else 0
                nw = n_tt - t0
                f, dg = fin2_prev, d_gen(n_qsb - 1, t0, n_tt)
                for _ in range(nw + 2):   # ysb, staging+recip, YnT first tile
                    next(f)
                for _k in range(nw):
                    next(dg)              # tt-k first half (reads qt-k cols)
                    try:
                        next(f)           # qt-(k+1)
                    except StopIteration:
                        pass
                    next(dg)              # tt-k second half
                next(dg)                  # window y DMA
                pull(10 ** 9)

    nc.compile()
    return nc


def make_in_maps(x, w_qkv, w_proj, T=T_FULL):
    """Per-core input dicts from full inputs (numpy), bf16-converted."""
    x = np.asarray(x, dtype=np.float32)
    w_qkv = np.asarray(w_qkv, dtype=np.float32).astype(NP_BF16)
    w_proj = np.asarray(w_proj, dtype=np.float32).astype(NP_BF16)
    cores_per_batch = N_CORES // B
    # x^T swizzled to [p, chunk, cchunk, tok] so each chunk DMA is one
    # contiguous run per partition (see XT in build_nc)
    n_ch = T // TOK_CHUNK
    xt_b = []
    for b in range(B):
        xt = x[b].T.reshape(CCHUNKS, 128, n_ch, TOK_CHUNK)
        xt_b.append(np.ascontiguousarray(
            xt.transpose(1, 2, 0, 3)).astype(NP_BF16))
    in_maps = []
    for core in range(N_CORES):
        b = core // cores_per_batch
        h0 = (core % cores_per_batch) * HEADS_PER_CORE
        h1, h2 = h0 + 1, h0 + 2
        col = lambda kind, h: w_qkv[:, kind * N_EMBD + h * HEAD_DIM:
                                    kind * N_EMBD + (h + 1) * HEAD_DIM]
        in_maps.append({
            "xt": xt_b[b],
            "wq01": np.ascontiguousarray(np.concatenate([col(0, h0), col(0, h1)], axis=1)),
            "wk01": np.ascontiguousarray(np.concatenate([col(1, h0), col(1, h1)], axis=1)),
            "wv01": np.ascontiguousarray(np.concatenate([col(2, h0), col(2, h1)], axis=1)),
            "wqv2": np.ascontiguousarray(np.concatenate([col(0, h2), col(2, h2)], axis=1)),
            "wk2": np.ascontiguousarray(col(1, h2)),
            "wp1": np.ascontiguousarray(w_proj[h0 * HEAD_DIM:(h1 + 1) * HEAD_DIM, :]),
            "wp2": np.ascontiguousarray(w_proj[h2 * HEAD_DIM:(h2 + 1) * HEAD_DIM, :]),
        })
    return in_maps


def gather_output(results, T=T_FULL):
    cores_per_batch = N_CORES // B
    out = np.empty((B, T, N_EMBD), dtype=np.float32)
    for b in range(B):
        parts = [np.asarray(results[b * cores_per_batch + j]["y"], dtype=np.float32)
                 for j in range(cores_per_batch)]
        out[b] = parts[0] + parts[1] + parts[2] + parts[3]
    return out


_CACHE = {}


def _get_nc(T=T_FULL, repeat=1):
    key = (T, repeat)
    if key not in _CACHE:
        _CACHE[key] = build_nc(T, repeat)
    return _CACHE[key]


def kernel(x, w_qkv, w_proj):
    import time as _time
    from concourse.bass_utils import run_bass_kernel_spmd
    T = x.shape[1]
    nc = _get_nc(T)
    in_maps = make_in_maps(x, w_qkv, w_proj, T)
    last_err = None
    for attempt in range(3):
        try:
            res = run_bass_kernel_spmd(nc, in_maps, list(range(N_CORES)))
            return gather_output(res.results, T)
        except Exception as e:  # transient device wedge: retry after a pause
            last_err = e
            _time.sleep(20 * (attempt + 1))
    raise last_err



# revision 44
# speedup vs baseline: 2.0081x; 1.1967x over previous
"""Causal self-attention (B=2, T=4096, C=768, H=12) on 8 trn2 NeuronCores.

Sharding: data-parallel on batch (cores 0-3 -> batch 0, cores 4-7 -> batch 1),
tensor-parallel on heads (3 heads per core).  Each core computes qkv for its
3 heads, causal flash-style attention, and a partial output projection
(its heads' rows of w_proj); the host sums the 4 partials per batch.

v11 structure (vs the serial-phase v7 baseline, ~2.05x faster: 676us ->
330us per iteration measured via repeat-differencing with block sampling):
- All activations/weights in bf16 (host-converted): halves DMA traffic and
  removes every fp32->fp32r rounding copy.  PSUM accumulation stays fp32.
  Partial Y outputs are written bf16 and summed fp32 on the host.
- Causal masking via gpsimd affine_select directly on the exp'd P tile
  (Pool engine is otherwise idle), freeing DVE; diagonal tiles compute
  S/PV ragged (columns left of the diagonal tile are skipped).
- One software-pipelined loop: the qkv projection chunk qs+1, V^T->V
  transposes, and the output projection for qs-1 are emitted interleaved
  into the attention rotation for query superblock qs, so their DMA/PE/
  DVE work hides under the attention inner loop (PE ~83% busy in sim).
- x^T is host-swizzled to [partition, chunk, cchunk, token] so each chunk
  DMA is one contiguous 6KB run per partition; y writes batch 4 token
  tiles per DMA.
"""

import sys

if '/opt/trn_rl_repo' not in sys.path:
    sys.path.insert(0, '/opt/trn_rl_repo')

from collections import deque

import numpy as np
import ml_dtypes

import concourse.bacc as bacc
import concourse.mybir as mybir
import concourse.tile as tile
from concourse.masks import make_identity

dt = mybir.dt
F32 = dt.float32
BF16 = dt.bfloat16
FP8 = dt.float8e4
NP_BF16 = ml_dtypes.bfloat16

# exp bias (in log space) applied to every attention logit before the fp8
# P tile: keeps exp(max_logit)+margin under the TRN fp8e4 max of 240 while
# keeping every row's max P far above the subnormal flush threshold.  The
# uniform scale cancels in the softmax normalization.
EXP_BIAS = -3.0 * float(np.log(2.0))

N_EMBD = 768
N_HEADS = 12
HEAD_DIM = 64
B = 2
T_FULL = 4096
N_CORES = 8
HEADS_PER_CORE = N_HEADS // (N_CORES // B)  # 3

TOK_CHUNK = 512   # qkv phase token chunk == query superblock
QSB = 512         # attention query superblock
KT = 128          # key tile (contraction for P@V)
CCHUNKS = N_EMBD // 128  # 6 contraction chunks


def build_nc(T=T_FULL, repeat=1, phases=('B', 'B2', 'C', 'D')):
    """Build the per-core Bass program.  Same program runs SPMD on all 8
    cores; per-core data (x^T of its batch, its heads' weight slices) comes
    via the input map.  `phases` subsets the per-iteration work (timing
    ablation only -- outputs are garbage unless all phases run)."""
    nc = bacc.Bacc(None, target_bir_lowering=False, debug=False)

    n_kt = T // KT
    n_qsb = T // QSB
    n_tok = T // 128
    kt_per_qsb = QSB // KT  # 4

    # x^T pre-swizzled on host to [p, chunk, cchunk, tok]: each chunk DMA
    # reads one contiguous 6KB run per partition.
    XT = nc.dram_tensor(
        "xt", [128, T // TOK_CHUNK, CCHUNKS, TOK_CHUNK], BF16,
        kind="ExternalInput")
    # per-head Q weights replicated across both 64-column halves: the qkv
    # matmul then yields Q_h on both partition halves, which is the moving
    # layout the block-diagonal K stationary needs for full-rate (K=128) S.
    WQ0R = nc.dram_tensor("wq0r", [N_EMBD, 128], BF16, kind="ExternalInput")
    WQ1R = nc.dram_tensor("wq1r", [N_EMBD, 128], BF16, kind="ExternalInput")
    WQ2R = nc.dram_tensor("wq2r", [N_EMBD, 128], BF16, kind="ExternalInput")
    WK01 = nc.dram_tensor("wk01", [N_EMBD, 128], BF16, kind="ExternalInput")
    WV01 = nc.dram_tensor("wv01", [N_EMBD, 128], BF16, kind="ExternalInput")
    WKV2 = nc.dram_tensor("wkv2", [N_EMBD, 128], BF16, kind="ExternalInput")
    WP1 = nc.dram_tensor("wp1", [128, N_EMBD], BF16, kind="ExternalInput")
    WP2 = nc.dram_tensor("wp2", [64, N_EMBD], BF16, kind="ExternalInput")
    Y = nc.dram_tensor("y", [T, N_EMBD], BF16, kind="ExternalOutput")

    xt_ap = XT.ap()

    with tile.TileContext(nc) as tc:
        with (
            tc.tile_pool(name="const", bufs=1) as const_pool,
            tc.tile_pool(name="wpool", bufs=1) as wpool,
            tc.tile_pool(name="qkvt", bufs=1) as qkvt,
            tc.tile_pool(name="vsb", bufs=1) as vsb_pool,
            tc.tile_pool(name="ynt", bufs=1) as ynt_pool,
            tc.tile_pool(name="xs", bufs=3) as xs_pool,
            tc.tile_pool(name="ptp", bufs=6) as pt_pool,
            tc.tile_pool(name="ysb", bufs=3) as ysb_pool,
            tc.tile_pool(name="fin", bufs=3) as fin_pool,
            tc.tile_pool(name="rp", bufs=8) as r_pool,
            tc.tile_pool(name="yout", bufs=3) as yout_pool,
            tc.tile_pool(name="yqn", bufs=4) as yqn_pool,
            tc.tile_pool(name="pbig", bufs=2, space="PSUM") as pbig,
            tc.tile_pool(name="py", bufs=2, space="PSUM") as py_pool,
            tc.tile_pool(name="paux", bufs=2, space="PSUM") as paux,
        ):
            # ---- weights: direct bf16 DMA (first, so phase B isn't gated
            # on constant construction; spread across two idle queues) ----
            _weng = [nc.gpsimd, nc.scalar]

            def load_w(src_ap, shape, tag, i=[0]):
                t = wpool.tile(shape, BF16, tag=tag)
                _weng[i[0] % 2].dma_start(out=t, in_=src_ap)
                i[0] += 1
                return t

            wq0rt = load_w(WQ0R.ap().rearrange("(c p) m -> p c m", p=128), [128, CCHUNKS, 128], "wq0rt")
            wq1rt = load_w(WQ1R.ap().rearrange("(c p) m -> p c m", p=128), [128, CCHUNKS, 128], "wq1rt")
            wq2rt = load_w(WQ2R.ap().rearrange("(c p) m -> p c m", p=128), [128, CCHUNKS, 128], "wq2rt")
            wk01r = load_w(WK01.ap().rearrange("(c p) m -> p c m", p=128), [128, CCHUNKS, 128], "wk01r")
            wv01r = load_w(WV01.ap().rearrange("(c p) m -> p c m", p=128), [128, CCHUNKS, 128], "wv01r")
            wkv2r = load_w(WKV2.ap().rearrange("(c p) m -> p c m", p=128), [128, CCHUNKS, 128], "wkv2r")
            wp1r = load_w(WP1.ap(), [128, N_EMBD], "wp1r")
            wp2r = load_w(WP2.ap(), [64, N_EMBD], "wp2r")

            # ---- constants ----
            ident_f = const_pool.tile([128, 128], F32)
            make_identity(nc, ident_f)
            identb = const_pool.tile([128, 128], BF16)
            nc.vector.tensor_copy(out=identb, in_=ident_f)
            bias_t = const_pool.tile([128, 1], F32)
            nc.vector.memset(bias_t, EXP_BIAS)

            # ---- persistent activations ----
            QR0 = qkvt.tile([128, T], BF16, tag="qr0")   # q_h0 on both halves
            QR1 = qkvt.tile([128, T], BF16, tag="qr1")
            QR2 = qkvt.tile([128, T], BF16, tag="qr2")
            KT01 = qkvt.tile([128, T], BF16, tag="kt01")
            VT01 = qkvt.tile([128, T], BF16, tag="vt01")
            KV2 = qkvt.tile([128, T], BF16, tag="kv2")   # k_h2 rows 0:64, v_h2 rows 64:128
            # block-diagonal K stationaries: KBD[:, h, kt] is the [128,128]
            # tile diag(K_h[:, 64-key half a], K_h[:, half b]) -- S then runs
            # with full 128 contraction (2.4x the K=64 rate on HW).  The
            # off-diagonal quadrants are zeroed once and never rewritten.
            KBD = qkvt.tile([128, HEADS_PER_CORE, n_kt, 128], BF16, tag="kbd")
            nc.vector.memset(
                KBD.rearrange("p a b c -> p (a b c)"), 0.0)
            # V in fp8, paired per DoubleRow k-tile: [keys, kt-pair, head,
            # j(2), 80] -- col 64 is the ones column (softmax denominator);
            # the 80-wide inner dim keeps the j-stride 16B-aligned as the
            # DoubleRow ldweights interleave requires.
            Vsb = vsb_pool.tile([128, n_kt // 2, HEADS_PER_CORE, 2, 80], FP8)
            # bf16 V copy for chunk 0 only: superblock 0's rows see few keys,
            # so fp8 P/V noise doesn't average out there -- those rows
            # (entirely contained in kt 0..3) take a bf16 PV path instead.
            Vsb0 = vsb_pool.tile([128, kt_per_qsb, HEADS_PER_CORE, 65], BF16,
                                 tag="vsb0")
            YnT01 = ynt_pool.tile([128, T], BF16, tag="ynt01")
            YnT2 = ynt_pool.tile([64, T], BF16, tag="ynt2")

            nc.vector.memset(
                Vsb[:, :, :, :, 64:65].rearrange("p a b c d -> p (a b c d)"),
                1.0)
            nc.vector.memset(
                Vsb0[:, :, :, 64:65].rearrange("p a b c -> p (a b c)"), 1.0)

            if phases != ('B', 'B2', 'C', 'D'):
                # timing-ablation build: zero every cross-phase tensor once so
                # skipped producers leave consumers with defined data
                for t in (QR0, QR1, QR2, KT01, VT01, KV2, YnT01):
                    nc.vector.memset(t, 0.0)
                nc.vector.memset(YnT2, 0.0)
                nc.vector.memset(
                    Vsb[:, :, :, :, 0:64].rearrange("p a b c d -> p (a b c) d"),
                    0.125)
                nc.vector.memset(
                    Vsb0[:, :, :, 0:64].rearrange("p a b c -> p (a b) c"), 0.125)

            qkv_jobs = [
                (wq0rt, QR0, 128), (wq1rt, QR1, 128), (wq2rt, QR2, 128),
                (wk01r, KT01, 128), (wv01r, VT01, 128), (wkv2r, KV2, 128),
            ]

            for _ in range(repeat):
                # ---------- work generators ----------
                def b_chunk_gen(ch, split_dma=False):
                    """qkv projection for token chunk ch ([512] tokens).
                    Yields every 2 contraction chunks so pulled side work
                    stays fine-grained (~1k PE cycles per step)."""
                    sl = slice(ch * TOK_CHUNK, (ch + 1) * TOK_CHUNK)
                    xs = xs_pool.tile([128, CCHUNKS, TOK_CHUNK], BF16)
                    if split_dma:
                        # halve time-to-first-matmul at program start
                        h = CCHUNKS // 2
                        nc.sync.dma_start(out=xs[:, 0:h], in_=xt_ap[:, ch, 0:h])
                        nc.sync.dma_start(out=xs[:, h:], in_=xt_ap[:, ch, h:])
                    else:
                        nc.sync.dma_start(out=xs, in_=xt_ap[:, ch])
                    yield
                    for wt, out_sb, m in qkv_jobs:
                        ps = paux.tile([128, TOK_CHUNK], F32, tag="aux")
                        for c in range(CCHUNKS):
                            nc.tensor.matmul(
                                ps[0:m, :], wt[:, c, 0:m], xs[:, c, :],
                                start=(c == 0), stop=(c == CCHUNKS - 1),
                            )
                            if c != CCHUNKS - 1:
                                yield
                        nc.vector.tensor_copy(out=out_sb[0:m, sl], in_=ps[0:m, :])
                        yield

                def b2_gen(ch):
                    """V^T -> V (keys-major) transposes for chunk ch's key
                    tiles (heads 0+1 ride one [128,128] transpose), plus the
                    block-diagonal K stationaries: diagonal quadrants that
                    stay on their partition half go via DVE, the shifted
                    ones via sbuf->sbuf DMA."""
                    for kt in range(ch * kt_per_qsb, (ch + 1) * kt_per_qsb):
                        ks = slice(kt * KT, (kt + 1) * KT)
                        ks0 = slice(kt * KT, kt * KT + 64)
                        ks1 = slice(kt * KT + 64, (kt + 1) * KT)
                        pv = paux.tile([128, 128], BF16, tag="aux")
                        nc.tensor.transpose(pv, VT01[:, ks], identb)
                        nc.vector.tensor_copy(
                            out=Vsb[:, kt // 2, 0:2, kt % 2, 0:64],
                            in_=pv.rearrange("p (b c) -> p b c", b=2))
                        if ch == 0:
                            nc.vector.tensor_copy(
                                out=Vsb0[:, kt, 0:2, 0:64],
                                in_=pv.rearrange("p (b c) -> p b c", b=2))
                        # h0: K dims live on partitions 0:64
                        nc.vector.tensor_copy(out=KBD[0:64, 0, kt, 0:64],
                                              in_=KT01[0:64, ks0])
                        nc.sync.dma_start(out=KBD[64:128, 0, kt, 64:128],
                                            in_=KT01[0:64, ks1])
                        # h1: K dims live on partitions 64:128
                        nc.sync.dma_start(out=KBD[0:64, 1, kt, 0:64],
                                            in_=KT01[64:128, ks0])
                        nc.vector.tensor_copy(out=KBD[64:128, 1, kt, 64:128],
                                              in_=KT01[64:128, ks1])
                        yield
                        pv2 = paux.tile([128, 64], BF16, tag="aux")
                        nc.tensor.transpose(pv2, KV2[64:128, ks], identb[64:128, 64:128])
                        nc.vector.tensor_copy(out=Vsb[:, kt // 2, 2, kt % 2, 0:64], in_=pv2)
                        if ch == 0:
                            nc.vector.tensor_copy(out=Vsb0[:, kt, 2, 0:64], in_=pv2)
                        # h2: K dims live on partitions 0:64 of KV2
                        nc.vector.tensor_copy(out=KBD[0:64, 2, kt, 0:64],
                                              in_=KV2[0:64, ks0])
                        nc.sync.dma_start(out=KBD[64:128, 2, kt, 64:128],
                                            in_=KV2[0:64, ks1])
                        yield

                def d_gen(qs, t0=0, t1=QSB // 128):
                    """partial output projection for superblock qs token
                    tiles [t0, t1).  Tiles stage into one buffer; for the
                    full window the write-back goes out in two DMAs so the
                    first overlaps the second half's matmuls."""
                    n_tt = QSB // 128
                    full = (t0 == 0 and t1 == n_tt)
                    yo = yout_pool.tile([128, n_tt, N_EMBD], BF16)
                    for tt4 in range(t0, t1):
                        tt = qs * n_tt + tt4
                        tsl = slice(tt * 128, (tt + 1) * 128)
                        for c0, ncols in ((0, 512), (512, 256)):
                            pp = paux.tile([128, 512], F32, tag="aux")
                            nc.tensor.matmul(pp[:, 0:ncols], YnT01[:, tsl],
                                             wp1r[:, c0:c0 + ncols], start=True, stop=False)
                            nc.tensor.matmul(pp[:, 0:ncols], YnT2[0:64, tsl],
                                             wp2r[0:64, c0:c0 + ncols], start=False, stop=True)
                            nc.vector.tensor_copy(out=yo[:, tt4, c0:c0 + ncols],
                                                  in_=pp[:, 0:ncols])
                            yield
                        if full and tt4 == n_tt // 2 - 1:
                            nc.sync.dma_start(
                                out=Y.ap()[qs * QSB:qs * QSB + QSB // 2, :]
                                    .rearrange("(tt p) c -> p tt c", p=128),
                                in_=yo[:, 0:n_tt // 2])
                    lo = qs * QSB + (QSB // 2 if full else t0 * 128)
                    nc.sync.dma_start(
                        out=Y.ap()[lo:qs * QSB + t1 * 128, :]
                            .rearrange("(tt p) c -> p tt c", p=128),
                        in_=yo[:, (n_tt // 2 if full else t0):t1])
                    yield

                # ---------- attention ----------
                heads_q = [QR0, QR1, QR2]

                def attend_kloop_gen0(h, yps):
                    """superblock 0: bf16 per-tile PV (low-context rows)."""
                    qt_h = heads_q[h]
                    nkt_q = kt_per_qsb
                    for kt2 in range(0, nkt_q, 2):
                        yield
                        last = (kt2 == nkt_q - 2)
                        q0 = QSB // 2 if last else 0
                        deltas = [(kt2 + j) * KT for j in range(2)]
                        q0s = [max(q0, min(d, QSB)) for d in deltas]
                        sps2 = pbig.tile([128, 2, QSB], F32, tag="big")
                        for j in range(2):
                            kt = kt2 + j
                            jsl = slice(q0s[j], QSB)
                            nc.tensor.matmul(sps2[:, j, jsl],
                                             KBD[:, h, kt, :],
                                             qt_h[:, q0s[j]:QSB],
                                             start=True, stop=True)
                        pt2 = pt_pool.tile([128, 2, QSB], BF16, tag="pt0")
                        for j in range(2):
                            jsl = slice(q0s[j], QSB)
                            nc.scalar.activation(
                                out=pt2[:, j, jsl], in_=sps2[:, j, jsl],
                                func=mybir.ActivationFunctionType.Exp,
                                scale=float(HEAD_DIM) ** -0.5, bias=bias_t,
                            )
                            nc.gpsimd.affine_select(
                                out=pt2[:, j, jsl], in_=pt2[:, j, jsl],
                                compare_op=mybir.AluOpType.is_ge,
                                fill=0.0, base=q0s[j] - deltas[j],
                                channel_multiplier=-1,
                                pattern=[[1, QSB - q0s[j]]],
                            )
                        for j in range(2):
                            kt = kt2 + j
                            jsl = slice(q0s[j], QSB)
                            nc.tensor.matmul(yps[:, jsl], Vsb0[:, kt, h, :],
                                             pt2[:, j, jsl],
                                             start=(kt == 0),
                                             stop=(kt == nkt_q - 1))

                def attend_kloop_gen(h, qs, qw0, qw1, yps):
                    """causal attention of head h for queries
                    [qs*QSB+qw0, qs*QSB+qw1) accumulated into
                    yps[:, qw0:qw1]."""
                    if qs == 0:
                        yield from attend_kloop_gen0(h, yps)
                        return
                    qt_h = heads_q[h]
                    nkt_w = (qs * QSB + qw1) // KT
                    # key tiles per exp group: keeps ~1024 elems/partition
                    # per activation instruction regardless of window width.
                    grp = 4 if qw1 - qw0 <= QSB // 2 else 2
                    assert nkt_w % 2 == 0

                    def s_group(kt2):
                        # diagonal tiles: query columns left of the tile's
                        # delta see none of its keys, so S runs ragged from
                        # max(qw0, delta).  Each PV pair is one fp8
                        # DoubleRow matmul over [q0p_pair:qw1]; stale pt2
                        # columns in [q0p:q0s[j]] are zeroed by the widened
                        # affine_select so they contribute nothing.
                        g = min(grp, nkt_w - kt2)
                        deltas = [(kt2 + j) * KT - qs * QSB for j in range(g)]
                        q0s = [max(qw0, min(d, qw1)) for d in deltas]
                        q0p = min(q0s)
                        sps = pbig.tile([128, g, qw1 - q0p], F32, tag="big")
                        for j in range(g):
                            kt = kt2 + j
                            nc.tensor.matmul(sps[:, j, q0s[j] - q0p:],
                                             KBD[:, h, kt, :],
                                             qt_h[:, qs * QSB + q0s[j]:qs * QSB + qw1],
                                             start=True, stop=True)
                        return (kt2, g, sps, deltas, q0s, q0p)

                    # S is emitted one group AHEAD of its exp/PV segment so
                    # the in-order PE queue always has the next S in flight
                    # before it reaches PV_k (which waits on exp_k) -- keeps
                    # the ACT exp stream gapless.
                    nxt = None
                    for kt2 in range(0, nkt_w, grp):
                        yield
                        if nxt is None:
                            nxt = s_group(0)
                        cur = nxt
                        nxt = s_group(kt2 + grp) if kt2 + grp < nkt_w else None
                        _, g, sps, deltas, q0s, q0p = cur
                        pt2 = pt_pool.tile([128, g, qw1 - q0p], FP8)
                        if all(q == q0p for q in q0s):
                            nc.scalar.activation(
                                out=pt2, in_=sps,
                                func=mybir.ActivationFunctionType.Exp,
                                scale=float(HEAD_DIM) ** -0.5, bias=bias_t,
                            )
                        else:
                            for j in range(g):
                                jsl = slice(q0s[j] - q0p, qw1 - q0p)
                                nc.scalar.activation(
                                    out=pt2[:, j, jsl], in_=sps[:, j, jsl],
                                    func=mybir.ActivationFunctionType.Exp,
                                    scale=float(HEAD_DIM) ** -0.5, bias=bias_t,
                                )
                        for j in range(g):
                            delta = deltas[j]
                            if delta + KT - 1 > q0s[j] or q0s[j] > q0p:
                                # keep P[i, idx] iff (q0p+idx) - i - delta >= 0
                                nc.gpsimd.affine_select(
                                    out=pt2[:, j, :], in_=pt2[:, j, :],
                                    compare_op=mybir.AluOpType.is_ge,
                                    fill=0.0, base=q0p - delta,
                                    channel_multiplier=-1,
                                    pattern=[[1, qw1 - q0p]],
                                )
                        for jj in range(0, g, 2):
                            q0pp = min(q0s[jj], q0s[jj + 1])
                            nc.tensor.matmul(
                                yps[:, q0pp:qw1],
                                Vsb[:, (kt2 + jj) // 2, h, :, 0:65],
                                pt2[:, jj:jj + 2, q0pp - q0p:],
                                start=(kt2 + jj == 0),
                                stop=(kt2 + jj == nkt_w - 2),
                                perf_mode=mybir.MatmulPerfMode.DoubleRow)

                def finish_gen(h, qs, yps, t0=0, t1=QSB // 128):
                    """transpose + normalize Y^T for (h, qs) token tiles
                    [t0, t1).  Tiles stage (transposed) into SBUF first so
                    one reciprocal serves the window."""
                    n_w = t1 - t0
                    ysb = ysb_pool.tile([65, QSB], BF16)
                    nc.vector.tensor_copy(out=ysb[:, t0 * 128:t1 * 128],
                                          in_=yps[:, t0 * 128:t1 * 128])
                    yield
                    st = fin_pool.tile([128, QSB // 128, 65], BF16)
                    for qt in range(t0, t1):
                        pt1 = paux.tile([128, 65], BF16, tag="aux")
                        nc.tensor.transpose(
                            pt1, ysb[:, qt * 128:(qt + 1) * 128], identb[0:65, 0:65])
                        nc.vector.tensor_copy(out=st[:, qt, :], in_=pt1)
                        if qt < t1 - 1:
                            yield
                    rr = r_pool.tile([128, QSB // 128], F32)
                    nc.vector.reciprocal(rr[:, t0:t1], st[:, t0:t1, 64])
                    yield
                    for qt in range(t0, t1):
                        csl = slice(qs * QSB + qt * 128, qs * QSB + (qt + 1) * 128)
                        yqn = yqn_pool.tile([128, 64], BF16)
                        nc.vector.tensor_scalar_mul(yqn, st[:, qt, 0:64],
                                                    rr[:, qt:qt + 1])
                        pt2r = paux.tile([64, 128], BF16, tag="aux")
                        nc.tensor.transpose(pt2r, yqn, identb)
                        if h == 0:
                            dst = YnT01[0:64, csl]
                        elif h == 1:
                            dst = YnT01[64:128, csl]
                        else:
                            dst = YnT2[0:64, csl]
                        nc.vector.tensor_copy(out=dst, in_=pt2r)
                        yield

                # ---------- interleaved schedule ----------
                side = deque()     # FIFO of generators (b/b2/d work)
                bwork = {}         # ch -> [gens] that must be emitted before
                                   # attention touches chunk ch

                def pull(n=1):
                    for _ in range(n):
                        while side:
                            try:
                                next(side[0])
                                break
                            except StopIteration:
                                side.popleft()
                        else:
                            return

                def drain(gens):
                    for g in gens:
                        for _ in g:
                            pass

                def drain_bwork_through(ch):
                    for c in range(ch + 1):
                        for g in bwork.pop(c, ()):
                            # may already be partially consumed via `side`
                            for _ in g:
                                pass

                def rotate(gens):
                    """gens: list of (gen, pulls_after_each_step).  Pulls
                    ride attention steps (ACT-paced, PE slack) rather than
                    finish steps so the exp stream stays dense."""
                    live = list(gens)
                    while live:
                        nxt = []
                        for g, npull in live:
                            try:
                                next(g)
                                nxt.append((g, npull))
                            except StopIteration:
                                pass
                            pull(npull)
                        live = nxt

                has = lambda p: p in phases
                # prologue: chunk 0 must be ready before attention qs=0;
                # chunk 1 is enqueued right away so each b(ch) has two full
                # superblocks of pull capacity before its forced drain.
                if has('B'):
                    drain([b_chunk_gen(0, split_dma=True)])
                if has('B2'):
                    drain([b2_gen(0)])
                if n_qsb > 1:
                    gens1 = ([b_chunk_gen(1)] if has('B') else []) + \
                            ([b2_gen(1)] if has('B2') else [])
                    bwork[1] = gens1
                    side.extend(gens1)

                if not has('C'):
                    for ch in range(1, n_qsb):
                        if has('B'):
                            drain([b_chunk_gen(ch)])
                        if has('B2'):
                            drain([b2_gen(ch)])
                    if has('D'):
                        for qs in range(n_qsb):
                            drain([d_gen(qs)])
                    continue

                fin2_prev = None   # finish gen of head 2 from previous qs
                for qs in range(n_qsb):
                    if qs + 2 < n_qsb:
                        gens = ([b_chunk_gen(qs + 2)] if has('B') else []) + \
                               ([b2_gen(qs + 2)] if has('B2') else [])
                        bwork[qs + 2] = gens
                        side.extend(gens)
                    # attention qs needs chunks <= qs fully emitted
                    drain_bwork_through(qs)

                    split_tail = has('D') and qs == n_qsb - 1 and n_qsb > 1
                    yps0 = py_pool.tile([65, QSB], F32, tag="y", name=f"yps0_{qs}")
                    yps1 = py_pool.tile([65, QSB], F32, tag="y", name=f"yps1_{qs}")
                    # for the last superblock, attention runs in two query
                    # half-windows so window A's finish+projection overlap
                    # window B's attention instead of serializing after it.
                    qw1 = QSB // 2 if split_tail else QSB
                    ht = qw1 // 128
                    g0 = attend_kloop_gen(0, qs, 0, qw1, yps0)
                    g1 = attend_kloop_gen(1, qs, 0, qw1, yps1)
                    rot_a = ([(fin2_prev, 1)] if fin2_prev is not None else []) \
                        + [(g0, 1), (g1, 1)]
                    rotate(rot_a)

                    if qs >= 1 and has('D'):
                        side.append(d_gen(qs - 1))
                    yps2 = py_pool.tile([65, QSB], F32, tag="y", name=f"yps2_{qs}")
                    g2 = attend_kloop_gen(2, qs, 0, qw1, yps2)
                    f0 = finish_gen(0, qs, yps0, 0, ht)
                    f1 = finish_gen(1, qs, yps1, 0, ht)
                    rotate([(f0, 0), (f1, 0), (g2, 1)])
                    fin2_prev = finish_gen(2, qs, yps2, 0, ht)

                    if split_tail:
                        # window B gets fresh yps tiles: a PSUM tile supports
                        # only one accumulation generation (one start=True
                        # group) in the tile framework.
                        yps0b = py_pool.tile([65, QSB], F32, tag="y", name="yps0b")
                        yps1b = py_pool.tile([65, QSB], F32, tag="y", name="yps1b")
                        g0b = attend_kloop_gen(0, qs, qw1, QSB, yps0b)
                        g1b = attend_kloop_gen(1, qs, qw1, QSB, yps1b)
                        rotate([(fin2_prev, 1), (g0b, 1), (g1b, 1)])
                        side.append(d_gen(qs, 0, ht))   # project window A
                        yps2b = py_pool.tile([65, QSB], F32, tag="y", name="yps2b")
                        g2b = attend_kloop_gen(2, qs, qw1, QSB, yps2b)
                        f0b = finish_gen(0, qs, yps0b, ht, QSB // 128)
                        f1b = finish_gen(1, qs, yps1b, ht, QSB // 128)
                        rotate([(f0b, 0), (f1b, 0), (g2b, 1)])
                        fin2_prev = finish_gen(2, qs, yps2b, ht, QSB // 128)

                # epilogue: lockstep head-2's last finish with the last
                # projection window (d tt-k needs fin2's qt-k written first)
                if not has('D'):
                    drain([fin2_prev])
                    pull(10 ** 9)
                    continue
                n_tt = QSB // 128
                t0 = n_tt // 2 if n_qsb > 1 else 0
                nw = n_tt - t0
                f, dg = fin2_prev, d_gen(n_qsb - 1, t0, n_tt)
                for _ in range(nw + 2):   # ysb, staging+recip, YnT first tile
                    next(f)
                for _k in range(nw):
                    next(dg)              # tt-k first half (reads qt-k cols)
                    try:
                        next(f)           # qt-(k+1)
                    except StopIteration:
                        pass
                    next(dg)              # tt-k second half
                next(dg)                  # window y DMA
                pull(10 ** 9)

    nc.compile()
    return nc


def make_in_maps(x, w_qkv, w_proj, T=T_FULL):
    """Per-core input dicts from full inputs (numpy), bf16-converted."""
    x = np.asarray(x, dtype=np.float32)
    w_qkv = np.asarray(w_qkv, dtype=np.float32).astype(NP_BF16)
    w_proj = np.asarray(w_proj, dtype=np.float32).astype(NP_BF16)
    cores_per_batch = N_CORES // B
    # x^T swizzled to [p, chunk, cchunk, tok] so each chunk DMA is one
    # contiguous run per partition (see XT in build_nc)
    n_ch = T // TOK_CHUNK
    xt_b = []
    for b in range(B):
        xt = x[b].T.reshape(CCHUNKS, 128, n_ch, TOK_CHUNK)
        xt_b.append(np.ascontiguousarray(
            xt.transpose(1, 2, 0, 3)).astype(NP_BF16))
    in_maps = []
    for core in range(N_CORES):
        b = core // cores_per_batch
        h0 = (core % cores_per_batch) * HEADS_PER_CORE
        h1, h2 = h0 + 1, h0 + 2
        col = lambda kind, h: w_qkv[:, kind * N_EMBD + h * HEAD_DIM:
                                    kind * N_EMBD + (h + 1) * HEAD_DIM]
        rep = lambda w: np.ascontiguousarray(np.concatenate([w, w], axis=1))
        in_maps.append({
            "xt": xt_b[b],
            "wq0r": rep(col(0, h0)),
            "wq1r": rep(col(0, h1)),
            "wq2r": rep(col(0, h2)),
            "wk01": np.ascontiguousarray(np.concatenate([col(1, h0), col(1, h1)], axis=1)),
            "wv01": np.ascontiguousarray(np.concatenate([col(2, h0), col(2, h1)], axis=1)),
            "wkv2": np.ascontiguousarray(np.concatenate([col(1, h2), col(2, h2)], axis=1)),
            "wp1": np.ascontiguousarray(w_proj[h0 * HEAD_DIM:(h1 + 1) * HEAD_DIM, :]),
            "wp2": np.ascontiguousarray(w_proj[h2 * HEAD_DIM:(h2 + 1) * HEAD_DIM, :]),
        })
    return in_maps


def gather_output(results, T=T_FULL):
    cores_per_batch = N_CORES // B
    out = np.empty((B, T, N_EMBD), dtype=np.float32)
    for b in range(B):
        parts = [np.asarray(results[b * cores_per_batch + j]["y"], dtype=np.float32)
                 for j in range(cores_per_batch)]
        out[b] = parts[0] + parts[1] + parts[2] + parts[3]
    return out


_CACHE = {}


def _get_nc(T=T_FULL, repeat=1):
    key = (T, repeat)
    if key not in _CACHE:
        _CACHE[key] = build_nc(T, repeat)
    return _CACHE[key]


def kernel(x, w_qkv, w_proj):
    import time as _time
    from concourse.bass_utils import run_bass_kernel_spmd
    T = x.shape[1]
    nc = _get_nc(T)
    in_maps = make_in_maps(x, w_qkv, w_proj, T)
    last_err = None
    for attempt in range(3):
        try:
            res = run_bass_kernel_spmd(nc, in_maps, list(range(N_CORES)))
            return gather_output(res.results, T)
        except Exception as e:  # transient device wedge: retry after a pause
            last_err = e
            _time.sleep(20 * (attempt + 1))
    raise last_err

